# revision 26
# baseline (speedup 1.0000x reference)
"""Fused additive-attention kernel for Trainium2 (8 NeuronCores, SPMD).

Computes  w = softmax_K( mask ? (Wl . tanh(vW_v^T + qW_q^T) + bl) : -1e9 )
without ever materializing the [B,N,S,K,H] joint_repr intermediate.

Sharding: data-parallel over batch B (16) across 8 cores -> 2 batches/core.
Weights replicated. Host does layout prep only (transposes / packing); all
FLOPs (matmuls, tanh, softmax) run on device.

Active-box packing: masked boxes contribute exactly 0 to the softmax, so the
host packs only the active boxes of each batch into Kpk = max_b(popcount)
slots (padded lanes get -1e9 logits via the mask trick) and scatters the
packed softmax back to K=50 positions afterwards.  All tanh/add/logit work
scales by Kpk/K.  The compiled kernel depends only on Kpk (cached; rebuilt
if an input's max active count changes) - it is correct for any box_mask.

Per-core dataflow over three phases of h-chunks [hc0], [hc1], [hc2,hc3]
(the first phase projects a single h-chunk so the roofline tanh stream
starts as early as possible; the last fuses two h-chunks so the DVE adds
amortize per-op overhead):
  qpT     [128, (b, d, ns)] bf16 per phase    (PE psum, d-alternating banks)
  VP2     duplicated-pair v-proj table (i, b, d, 2), bv+bq folded in
  JT      [128, (j, strip, b, d, ns)] bf16 = QPs + vp:  ONE DVE
          tensor_tensor per (j, strip) covering all (b, d) via the merged
          bd AP dim (pair APs -> 2x mode; ~0.65ns/elem measured)
  tanh    in-place on JT, one ACT op per j-group (ACT at 1 elem/cycle/lane
          is the roofline: 65536 cols -> ~55 us at Kpk=32)
  logits  psum [32+kh, 512]: rows 0:kh strip 0 / 32:32+kh strip 1, shared
          zero-padded Wl lhsT, adjacent matmuls via tile_position row 32.
  masked softmax over packed lanes after PE-transposing logits to [ns, i];
  output leaves in native [128, (nsc, i)] layout, host de-interleaves.
"""

import os
import sys

import numpy as np

sys.path.insert(0, "/opt/trn_rl_repo")

import concourse.bass as bass
import concourse.mybir as mybir
from concourse import bacc, bass_utils
from concourse.tile import TileContext

# Problem shapes (hardcoded per contract -- kernel.py must be self-contained)
B, N, S, K = 16, 4, 64, 50
VD, QD, H = 1024, 768, 512
NCORES = 8
BPC = B // NCORES          # batches per core = 2
NS = BPC * N * S           # 512 rows (b, n, s) per core
HC = H // 128              # 4 h-chunks
QC = QD // 128             # 6 qd-chunks
VC = VD // 128             # 8 vd-chunks
NSB = NS // BPC            # 256 (n,s) columns per batch

PHASES = [[0], [1], [2, 3]]            # h-chunks per phase

F32 = mybir.dt.float32
BF16 = mybir.dt.bfloat16

_CACHE = {}


def _groups(kh, kind):
    """Split range(kh) into j-groups.

    kind 0 (first phase): tiny leading groups so the first tanh issues
    early.  kind 2 (last phase): small trailing groups so the post-tanh
    logit-matmul tail is short.  kind 1: plain mid-size groups.
    """
    if kind == 0:
        pattern = [2, 2, 4, 4, 4]
    elif kind == 1:
        pattern = [8, 8]
    else:
        pattern = [4, 6, 4, 1, 1]
    sizes, rem = [], kh
    for s in pattern:
        s = min(s, rem)
        if s <= 0:
            break
        sizes.append(s)
        rem -= s
    while rem > 0:
        s = min(6, rem)
        sizes.append(s)
        rem -= s
    out, at = [], 0
    for s in sizes:
        out.append(list(range(at, at + s)))
        at += s
    return out


def _bnd_layout(kpk):
    """Column layout of the bf16 input bundles (per-partition views)."""
    kb2 = BPC * kpk
    # bundle A: (wqtA_qc | qts_qc) x 6, then vts, then wvtA
    xa = QC * (256 + NS) + VC * kb2 + VC * 256
    # bundle B: wqtB | wvtB | wlz
    kh = kpk // 2
    xb = QC * 256 + VC * 256 + HC * kh * kh
    return xa, xb


def _build_nc(kpk):
    kh = kpk // 2              # strip width (psum rows 0:kh and 32:32+kh)
    kb2 = BPC * kpk            # packed (b, i) columns per core
    xa, xb = _bnd_layout(kpk)
    xf = 12 + 2 * kb2 + 128    # f32 bundle: wlb | msk | ident

    nc = bacc.Bacc("TRN2", target_bir_lowering=False)

    xa1 = 2 * (256 + NS)
    xa2 = (QC - 2) * (256 + NS)
    xa3 = xa - xa1 - xa2
    bndA1_h = nc.dram_tensor("bndA1", [128, xa1], BF16, kind="ExternalInput")
    bndA2_h = nc.dram_tensor("bndA2", [128, xa2], BF16, kind="ExternalInput")
    bndA3_h = nc.dram_tensor("bndA3", [128, xa3], BF16, kind="ExternalInput")
    bndB_h = nc.dram_tensor("bndB", [128, xb], BF16, kind="ExternalInput")
    bndF_h = nc.dram_tensor("bndF", [128, xf], F32, kind="ExternalInput")
    out_h = nc.dram_tensor("out", [128, (NS // 128) * kpk], F32,
                           kind="ExternalOutput")

    # phase geometry
    nph = len(PHASES)
    nds = [len(ds) for ds in PHASES]
    qoff = [sum(nds[:p]) * NS for p in range(nph)]          # QPs col offsets
    voff = [sum(nds[:p]) * 2 * kb2 for p in range(nph)]     # VP2 col offsets

    with TileContext(nc) as tc:
        with (
            tc.tile_pool(name="persist", bufs=1) as pp,
            tc.tile_pool(name="ppsum", bufs=1, space="PSUM") as ppsum,
            tc.tile_pool(name="smpsum", bufs=2, space="PSUM") as sps,
        ):
            # ---- bundle loads: 5 flat DMAs, ramp-critical slices first ----
            bndA = pp.tile([128, xa], BF16, name="bndA")
            c2 = 2 * (256 + NS)            # qc chunks 0-1
            nc.sync.dma_start(bndA[:, QC * (256 + NS) :], bndA3_h[:, :])
            nc.sync.dma_start(bndA[:, 0:c2], bndA1_h[:, :])
            nc.sync.dma_start(bndA[:, c2 : QC * (256 + NS)], bndA2_h[:, :])
            bndF = pp.tile([128, xf], F32, name="bndF")
            nc.sync.dma_start(bndF[:, :], bndF_h[:, :])
            bndB = pp.tile([128, xb], BF16, name="bndB")
            nc.sync.dma_start(bndB[:, :], bndB_h[:, :])

            def wq_hc(qc, hc):
                off = (hc % 2) * 128
                if hc < 2:
                    base = qc * (256 + NS)
                    return bndA[:, base + off : base + off + 128]
                return bndB[:, qc * 256 + off : qc * 256 + off + 128]

            def qts_qc(qc):
                base = qc * (256 + NS) + 256
                return bndA[:, base : base + NS]

            def vts_vc(vc):
                base = QC * (256 + NS) + vc * kb2
                return bndA[:, base : base + kb2]

            def wv_hc(vc, hc):
                off = vc * 256 + (hc % 2) * 128
                if hc < 2:
                    base = QC * (256 + NS) + VC * kb2
                    return bndA[:, base + off : base + off + 128]
                return bndB[:, QC * 256 + off : QC * 256 + off + 128]

            # PE warmup: ramp the tensor-engine clock while DMAs land
            # (memset on the otherwise-idle Pool engine)
            warm = pp.tile([128, 256], BF16, name="warm")
            nc.gpsimd.memset(warm[:, :], 0.0)
            with tc.tile_pool(name="warmps", bufs=1, space="PSUM") as wps:
                pw = wps.tile([128, 256], F32, name="pw")
                for _ in range(10):
                    nc.tensor.matmul(pw[:, :], warm[:, 0:128], warm[:, :],
                                     start=True, stop=True)

            wlz0 = QC * 256 + VC * 256
            wlb = bndF[:, 0:12]
            msk = bndF[:, 12 : 12 + 2 * kb2]
            ident = bndF[:, 12 + 2 * kb2 :]

            # qpT: [128, (ph, b, d, ns)] bf16 (d = hc within phase)
            QPs = pp.tile([128, HC * NS], BF16, name="QPs")
            # duplicated-pair v-proj table: [128, (ph, i, b, d, 2)] bf16
            VP2 = pp.tile([128, HC * kb2 * 2], BF16, name="VP2")

            # logits psum: rows 0:kh <- strip 0, rows 32:32+kh <- strip 1
            ps_log = ppsum.tile([32 + kh, NS], F32, name="ps_log")

            proj_state = {}

            def proj_mm(ph):
                """PE half of phase-ph projection (v-proj first: its tiny
                matmuls burn the tensor engine's slow-clock window)."""
                ds = PHASES[ph]
                nd = len(ds)
                pool_cm = tc.tile_pool(name=f"p1ps{ph}", bufs=1, space="PSUM")
                p1ps = pool_cm.__enter__()
                pv = [p1ps.tile([128, kb2], F32, tag=f"pv{d}",
                                name="pv") for d in range(nd)]
                for vc in range(VC):
                    for d in range(nd):
                        nc.tensor.matmul(
                            pv[d][:, :],
                            wv_hc(vc, ds[d]),
                            vts_vc(vc),
                            start=(vc == 0),
                            stop=(vc == VC - 1),
                        )
                pq = [p1ps.tile([128, NS], F32, tag=f"pq{d}", name="pq")
                      for d in range(nd)]
                for qc in range(QC):
                    for d in range(nd):
                        nc.tensor.matmul(
                            pq[d][:, :],
                            wq_hc(qc, ds[d]),
                            qts_qc(qc),
                            start=(qc == 0),
                            stop=(qc == QC - 1),
                        )
                proj_state[ph] = (pool_cm, pv, pq)

            def vp2_copy(ph):
                """DVE: pv psum -> pair-duplicated VP2 (i, b, d, 2)."""
                ds = PHASES[ph]
                nd = len(ds)
                vp_v = VP2[:, voff[ph] : voff[ph] + nd * 2 * kb2].rearrange(
                    "p (i b dd two) -> p i b dd two", i=kpk, b=BPC, dd=nd
                )
                _, pv, _ = proj_state[ph]
                for d in range(nd):
                    hc = ds[d]
                    nc.vector.tensor_scalar_add(
                        vp_v[:, :, :, d : d + 1, :],
                        pv[d][:, :]
                        .rearrange("p (b i one two) -> p i b one two",
                                   b=BPC, one=1, two=1)
                        .broadcast_to((128, kpk, BPC, 1, 2)),
                        wlb[:, 2 * HC + hc : 2 * HC + hc + 1],
                    )

            def qps_copy(ph):
                """DVE: pq psum -> QPs bf16 (bq folded into VP2's bias);
                closes the phase's projection psum pool."""
                ds = PHASES[ph]
                nd = len(ds)
                qp_v = QPs[:, qoff[ph] : qoff[ph] + nd * NS].rearrange(
                    "p (b dd x) -> p b dd x", b=BPC, dd=nd
                )
                pool_cm, pv, pq = proj_state.pop(ph)
                for d in range(nd):
                    nc.vector.tensor_copy(
                        qp_v[:, :, d : d + 1, :],
                        pq[d][:, :].rearrange(
                            "p (b one x) -> p b one x", b=BPC, one=1
                        ),
                    )
                pool_cm.__exit__(None, None, None)

            def proj_copy(ph):
                vp2_copy(ph)
                qps_copy(ph)

            def main_phase(ph, mp, mid_cb=None, late_cb=None):
                """Joint tanh + logit matmuls for one phase's h-chunks."""
                ds = PHASES[ph]
                nd = len(ds)
                slabw = 2 * nd * NSB            # (b, d, x) cols per (j, strip)
                kind = 0 if ph == 0 else (2 if ph == nph - 1 else 1)
                groups = _groups(kh, kind)
                qp_ph = QPs[:, qoff[ph] : qoff[ph] + nd * NS]
                late_at = len(groups) - 1
                # phase 0: the first group reads the projection psum
                # directly (1x DVE mode) so the first tanh does not wait
                # for the QPs copies; the copies land at the g==1 boundary
                # (before the next phase's psum pool opens, keeping pool
                # LIFO order)
                psum_groups = 0
                mid_at = min(2, len(groups) - 1)
                for g, js in enumerate(groups):
                    if ph == 0 and g == 0:
                        qps_copy(0)
                    if g == mid_at and mid_cb is not None:
                        mid_cb()
                    if g == late_at and late_cb is not None:
                        late_cb()
                    L = len(js)
                    JT = mp.tile([128, L * 2 * slabw], BF16, tag="JT",
                                 name="JT")
                    for jj, j in enumerate(js):
                        for strip in range(2):
                            i0 = j + strip * kh
                            sb = (jj * 2 + strip) * slabw
                            vo = voff[ph] + i0 * 4 * nd
                            if g < psum_groups:
                                src0 = proj_state[0][2][0][:, :].rearrange(
                                    "p (bd xh two) -> p bd xh two",
                                    bd=2 * nd, xh=128,
                                )
                            else:
                                src0 = qp_ph.rearrange(
                                    "p (bd xh two) -> p bd xh two",
                                    bd=2 * nd, xh=128,
                                )
                            nc.vector.tensor_add(
                                JT[:, sb : sb + slabw].rearrange(
                                    "p (bd xh two) -> p bd xh two",
                                    bd=2 * nd, xh=128,
                                ),
                                src0,
                                VP2[:, vo : vo + 4 * nd]
                                .rearrange("p (bd one two) -> p bd one two",
                                           bd=2 * nd, one=1)
                                .broadcast_to((128, 2 * nd, 128, 2)),
                            )
                    # in-place tanh over the whole group
                    nc.scalar.activation(
                        JT[:, :], JT[:, :], mybir.ActivationFunctionType.Tanh
                    )
                    for jj, j in enumerate(js):
                        for d in range(nd):
                            hc = ds[d]
                            first = ph == 0 and g == 0 and jj == 0 and d == 0
                            last = (
                                ph == nph - 1
                                and g == len(groups) - 1
                                and jj == L - 1
                                and d == nd - 1
                            )
                            wl_col = bndB[
                                :,
                                wlz0 + hc * kh * kh + j * kh
                                : wlz0 + hc * kh * kh + (j + 1) * kh,
                            ]
                            for strip in range(2):
                                sb = (jj * 2 + strip) * slabw
                                rhs = JT[:, sb : sb + slabw].rearrange(
                                    "p (b dd x) -> p b dd x", b=BPC, dd=nd
                                )[:, :, d : d + 1, :]
                                nc.tensor.matmul(
                                    ps_log[32 * strip : 32 * strip + kh, :],
                                    wl_col,
                                    rhs,
                                    start=first,
                                    stop=last,
                                    tile_position=(0, 32 * strip),
                                    skip_group_check=True,
                                )

            def mk_mm_cb(p):
                def cb():
                    proj_mm(p)
                return cb

            def mk_copy_cb(p):
                def cb():
                    proj_copy(p)
                return cb

            proj_mm(0)
            vp2_copy(0)
            with tc.tile_pool(name="main", bufs=4) as mp:
                for ph in range(nph):
                    nxt = ph + 1 if ph + 1 < nph else None
                    main_phase(
                        ph, mp,
                        mid_cb=mk_mm_cb(nxt) if nxt is not None else None,
                        late_cb=mk_copy_cb(nxt) if nxt is not None else None,
                    )

            # ---- masked softmax over packed lanes ----
            LG0 = pp.tile([kh, NS], F32, name="LG0")
            LG1 = pp.tile([32 + kh, NS], F32, name="LG1")
            W_all = pp.tile([128, NS // 128, kpk], F32, name="W_all")
            nc.vector.tensor_copy(LG0[:, :], ps_log[0:kh, :])
            nc.vector.tensor_copy(LG1[32 : 32 + kh, :], ps_log[32 : 32 + kh, :])
            for nsc in range(NS // 128):
                b = nsc // (NSB // 128)
                LT = pp.tile([128, kpk], F32, name=f"LT{nsc}")
                for half in range(2):
                    ps_t = sps.tile([128, kh], F32, tag="ps_t", name="ps_t")
                    if half == 0:
                        src = LG0[0:kh, nsc * 128 : (nsc + 1) * 128]
                        idn = ident[0:kh, 0:kh]
                    else:
                        src = LG1[32 : 32 + kh, nsc * 128 : (nsc + 1) * 128]
                        idn = ident[32 : 32 + kh, 32 : 32 + kh]
                    nc.tensor.transpose(ps_t[:, :], src, idn)
                    nc.vector.tensor_copy(
                        LT[:, half * kh : (half + 1) * kh], ps_t[:, :]
                    )
                # masked = logits*validf + (validf-1)*1e9
                nc.vector.tensor_mul(
                    LT[:, :], LT[:, :], msk[:, b * kpk : (b + 1) * kpk]
                )
                nc.vector.tensor_add(
                    LT[:, :], LT[:, :], msk[:, kb2 + b * kpk : kb2 + (b + 1) * kpk]
                )
                # |logit| <= sum|Wl|*1 ~ 1.3, so exp needs no max-shift;
                # masked lanes are -1e9 -> exp underflows to exactly 0.
                EX = pp.tile([128, kpk], F32, name=f"EX{nsc}")
                sm = pp.tile([128, 1], F32, name=f"sm{nsc}")
                nc.scalar.activation(
                    EX[:, :], LT[:, :], mybir.ActivationFunctionType.Exp,
                    accum_out=sm[:, 0:1],
                )
                rs = pp.tile([128, 1], F32, name=f"rs{nsc}")
                nc.vector.reciprocal(rs[:, :], sm[:, :])
                nc.vector.tensor_scalar_mul(
                    W_all[:, nsc, :], EX[:, :], rs[:, 0:1]
                )
            # single output DMA: per-partition contiguous 512B descriptors
            # (per-nsc slices would cost 4x the descriptor count)
            nc.sync.dma_start(out_h[:, :], W_all[:, :, :])

    nc.finalize()
    return nc


def _prep_in_maps(v, q, box_mask, Wv, bv, Wq, bq, Wl, kpk, active):
    """Host-side layout prep: shard over B, pack active boxes, bundle."""
    import ml_dtypes

    kh = kpk // 2
    kb2 = BPC * kpk
    xa, xb = _bnd_layout(kpk)

    v = np.asarray(v, np.float32).reshape(B, K, VD)
    q = np.asarray(q, np.float32).reshape(B, N * S, QD)

    # packed v + validity per batch
    vpk = np.zeros((B, kpk, VD), np.float32)
    valid = np.zeros((B, kpk), np.float32)
    for b in range(B):
        kb = len(active[b])
        vpk[b, :kb] = v[b, active[b]]
        valid[b, :kb] = 1.0

    WqT = np.asarray(Wq, np.float32).T                                # [QD, H]
    WvT = np.asarray(Wv, np.float32).T                                # [VD, H]
    wl_chunks = np.asarray(Wl, np.float32).reshape(4, 128).T          # [128, hc]
    bvq = np.asarray(bv, np.float32) + np.asarray(bq, np.float32)

    # per-partition chunked views [128, C, x]
    def chunked(mT, width):  # mT [rows=c*128, width]
        c = mT.shape[0] // 128
        return mT.reshape(c, 128, width).transpose(1, 0, 2)           # [128,c,w]

    WqA = chunked(WqT[:, :256], 256)                                  # [128,6,256]
    WqB = chunked(WqT[:, 256:], 256)
    WvA = chunked(WvT[:, :256], 256)                                  # [128,8,256]
    WvB = chunked(WvT[:, 256:], 256)

    wlz = np.zeros((128, HC, kh, kh), np.float32)
    for j in range(kh):
        wlz[:, :, j, j] = wl_chunks
    wlz = wlz.reshape(128, HC * kh * kh)

    wlb = np.zeros((128, 12), np.float32)
    wlb[:, 0:4] = wl_chunks
    wlb[:, 8:12] = bvq.reshape(4, 128).T

    bndB = np.zeros((128, xb), np.float32)
    bndB[:, : QC * 256] = WqB.reshape(128, QC * 256)
    bndB[:, QC * 256 : QC * 256 + VC * 256] = WvB.reshape(128, VC * 256)
    bndB[:, QC * 256 + VC * 256 :] = wlz
    bndB = bndB.astype(ml_dtypes.bfloat16)

    ident = np.eye(128, dtype=np.float32)

    in_maps = []
    for c in range(NCORES):
        b0 = c * BPC
        qc = q[b0 : b0 + BPC].reshape(NS, QD)
        qTc = chunked(np.ascontiguousarray(qc.T), NS)                 # [128,6,NS]
        vc = vpk[b0 : b0 + BPC].reshape(kb2, VD)
        vTc = chunked(np.ascontiguousarray(vc.T), kb2)                # [128,8,kb2]

        bndA = np.zeros((128, xa), np.float32)
        for qq in range(QC):
            base = qq * (256 + NS)
            bndA[:, base : base + 256] = WqA[:, qq]
            bndA[:, base + 256 : base + 256 + NS] = qTc[:, qq]
        vo = QC * (256 + NS)
        bndA[:, vo : vo + VC * kb2] = vTc.reshape(128, VC * kb2)
        bndA[:, vo + VC * kb2 :] = WvA.reshape(128, VC * 256)
        bndA = bndA.astype(ml_dtypes.bfloat16)
        xa1 = 2 * (256 + NS)
        xa2 = (QC - 2) * (256 + NS)

        mf = valid[b0 : b0 + BPC].reshape(1, kb2)
        bndF = np.zeros((128, 12 + 2 * kb2 + 128), np.float32)
        bndF[:, 0:12] = wlb
        bndF[:, 12 : 12 + kb2] = mf
        bndF[:, 12 + kb2 : 12 + 2 * kb2] = (mf - 1.0) * 1e9
        bndF[:, 12 + 2 * kb2 :] = ident

        in_maps.append({
            "bndA1": np.ascontiguousarray(bndA[:, :xa1]),
            "bndA2": np.ascontiguousarray(bndA[:, xa1 : xa1 + xa2]),
            "bndA3": np.ascontiguousarray(bndA[:, xa1 + xa2 :]),
            "bndB": bndB,
            "bndF": bndF,
        })
    return in_maps


def kernel(v, q, box_mask, tags_attention, Wv, bv, Wq, bq, Wl, bl):
    # bl shifts all unmasked logits uniformly -> cancels in softmax.
    # tags_attention is unused by the reference module.
    bm = np.asarray(box_mask).reshape(B, K)
    active = [np.nonzero(bm[b] > 0)[0] for b in range(B)]
    kmax = max(len(a) for a in active)
    if kmax == 0:
        # every box masked in every batch: reference softmax is uniform
        return np.full((B, N, S, K), 1.0 / K, np.float32)
    kpk = max(2, kmax + (kmax & 1))       # even, >= 2

    if _CACHE.get("kpk") != kpk:
        _CACHE["nc"] = _build_nc(kpk)
        _CACHE["kpk"] = kpk
    nc = _CACHE["nc"]
    in_maps = _prep_in_maps(v, q, box_mask, Wv, bv, Wq, bq, Wl, kpk, active)
    res = bass_utils.run_bass_kernel_spmd(
        nc,
        in_maps,
        core_ids=list(range(NCORES)),
        trace=bool(os.environ.get("KERNEL_TRACE")),
        tmpdir=os.environ.get("KERNEL_TMPDIR"),
    )
    _CACHE["last_result"] = res
    w = np.zeros((B, N, S, K), np.float32)
    for c in range(NCORES):
        # out [128, (nsc, i)] -> rows (nsc*128+p) = (b, n, s) order
        wo = res.results[c]["out"].reshape(128, NS // 128, kpk)
        wp = wo.transpose(1, 0, 2).reshape(BPC, N, S, kpk)
        for bi in range(BPC):
            b = c * BPC + bi
            kb = len(active[b])
            if kb == 0:
                w[b] = 1.0 / K          # all-masked row: uniform softmax
            else:
                w[b][:, :, active[b]] = wp[bi][:, :, :kb]
    return w


# revision 27
# speedup vs baseline: 1.1807x; 1.1807x over previous
"""Fused additive-attention kernel for Trainium2 (8 NeuronCores, SPMD).

Computes  w = softmax_K( mask ? (Wl . tanh(vW_v^T + qW_q^T) + bl) : -1e9 )
without ever materializing the [B,N,S,K,H] joint_repr intermediate.

Sharding: data-parallel over batch B (16) across 8 cores -> 2 batches/core.
Weights replicated. Host does layout prep only (transposes / packing); all
FLOPs (matmuls, tanh, softmax) run on device.

Active-box packing: masked boxes contribute exactly 0 to the softmax, so the
host packs only the active boxes of each batch into Kpk = max_b(popcount)
slots (padded lanes get -1e9 logits via the mask trick) and scatters the
packed softmax back to K=50 positions afterwards.  All tanh/add/logit work
scales by Kpk/K.  The compiled kernel depends only on Kpk (cached; rebuilt
if an input's max active count changes) - it is correct for any box_mask.

Per-core dataflow over three phases of h-chunks [hc0], [hc1], [hc2,hc3]
(the first phase projects a single h-chunk so the roofline tanh stream
starts as early as possible; the last fuses two h-chunks so the DVE adds
amortize per-op overhead):
  qpT     [128, (b, d, ns)] bf16 per phase    (PE psum, d-alternating banks)
  VP2     duplicated-pair v-proj table (i, b, d, 2), bv+bq folded in
  JT      [128, (j, strip, b, d, ns)] bf16 = QPs + vp:  ONE DVE
          tensor_tensor per (j, strip) covering all (b, d) via the merged
          bd AP dim (pair APs -> 2x mode; ~0.65ns/elem measured)
  tanh    in-place on JT, one ACT op per j-group (ACT at 1 elem/cycle/lane
          is the roofline: 65536 cols -> ~55 us at Kpk=32)
  logits  psum [32+kh, 512]: rows 0:kh strip 0 / 32:32+kh strip 1, shared
          zero-padded Wl lhsT, adjacent matmuls via tile_position row 32.
  masked softmax over packed lanes after PE-transposing logits to [ns, i];
  output leaves in native [128, (nsc, i)] layout, host de-interleaves.
"""

import os
import sys

import numpy as np

sys.path.insert(0, "/opt/trn_rl_repo")

import concourse.bass as bass
import concourse.mybir as mybir
from concourse import bacc, bass_utils
from concourse.tile import TileContext

# Problem shapes (hardcoded per contract -- kernel.py must be self-contained)
B, N, S, K = 16, 4, 64, 50
VD, QD, H = 1024, 768, 512
NCORES = 8
BPC = B // NCORES          # batches per core = 2
NS = BPC * N * S           # 512 rows (b, n, s) per core
HC = H // 128              # 4 h-chunks
QC = QD // 128             # 6 qd-chunks
VC = VD // 128             # 8 vd-chunks
NSB = NS // BPC            # 256 (n,s) columns per batch

PHASES = [[0], [1], [2, 3]]            # h-chunks per phase

F32 = mybir.dt.float32
BF16 = mybir.dt.bfloat16

_CACHE = {}


def _groups(kh, kind):
    """Split range(kh) into j-groups.

    kind 0 (first phase): tiny leading groups so the first tanh issues
    early.  kind 2 (last phase): small trailing groups so the post-tanh
    logit-matmul tail is short.  kind 1: plain mid-size groups.
    """
    if kind == 0:
        pattern = [2, 2, 2, 4, 4, 2]
    elif kind == 1:
        pattern = [6, 6, 4]
    else:
        pattern = [4, 6, 2, 2, 1, 1]
    sizes, rem = [], kh
    for s in pattern:
        s = min(s, rem)
        if s <= 0:
            break
        sizes.append(s)
        rem -= s
    while rem > 0:
        s = min(6, rem)
        sizes.append(s)
        rem -= s
    out, at = [], 0
    for s in sizes:
        out.append(list(range(at, at + s)))
        at += s
    return out


def _bnd_layout(kpk):
    """Column layout of the bf16 input bundles (per-partition views)."""
    kb2 = BPC * kpk
    # bundle A: (wqtA_qc | qts_qc) x 6, then vts, then wvtA
    xa = QC * (256 + NS) + VC * kb2 + VC * 256
    # bundle B: wqtB | wvtB | wlz
    kh = kpk // 2
    xb = QC * 256 + VC * 256 + HC * kh * kh
    return xa, xb


def _build_nc(kpk):
    kh = kpk // 2              # strip width (psum rows 0:kh and 32:32+kh)
    kb2 = BPC * kpk            # packed (b, i) columns per core
    xa, xb = _bnd_layout(kpk)
    xf = 12 + 2 * kb2 + 128    # f32 bundle: wlb | msk | ident

    nc = bacc.Bacc("TRN2", target_bir_lowering=False)

    xa1 = 2 * (256 + NS)
    xa2 = (QC - 2) * (256 + NS)
    xa3 = xa - xa1 - xa2
    bndA1_h = nc.dram_tensor("bndA1", [128, xa1], BF16, kind="ExternalInput")
    bndA2_h = nc.dram_tensor("bndA2", [128, xa2], BF16, kind="ExternalInput")
    bndA3_h = nc.dram_tensor("bndA3", [128, xa3], BF16, kind="ExternalInput")
    bndB_h = nc.dram_tensor("bndB", [128, xb], BF16, kind="ExternalInput")
    bndF_h = nc.dram_tensor("bndF", [128, xf], F32, kind="ExternalInput")
    out_h = nc.dram_tensor("out", [128, (NS // 128) * kpk], F32,
                           kind="ExternalOutput")

    # phase geometry
    nph = len(PHASES)
    nds = [len(ds) for ds in PHASES]
    qoff = [sum(nds[:p]) * NS for p in range(nph)]          # QPs col offsets
    voff = [sum(nds[:p]) * 2 * kb2 for p in range(nph)]     # VP2 col offsets

    with TileContext(nc) as tc:
        with (
            tc.tile_pool(name="persist", bufs=1) as pp,
            tc.tile_pool(name="ppsum", bufs=1, space="PSUM") as ppsum,
            tc.tile_pool(name="smpsum", bufs=2, space="PSUM") as sps,
        ):
            # ---- bundle loads: 5 flat DMAs, ramp-critical slices first ----
            bndA = pp.tile([128, xa], BF16, name="bndA")
            c2 = 2 * (256 + NS)            # qc chunks 0-1
            nc.sync.dma_start(bndA[:, QC * (256 + NS) :], bndA3_h[:, :])
            nc.sync.dma_start(bndA[:, 0:c2], bndA1_h[:, :])
            nc.sync.dma_start(bndA[:, c2 : QC * (256 + NS)], bndA2_h[:, :])
            bndF = pp.tile([128, xf], F32, name="bndF")
            nc.sync.dma_start(bndF[:, :], bndF_h[:, :])
            bndB = pp.tile([128, xb], BF16, name="bndB")
            nc.sync.dma_start(bndB[:, :], bndB_h[:, :])

            def wq_hc(qc, hc):
                off = (hc % 2) * 128
                if hc < 2:
                    base = qc * (256 + NS)
                    return bndA[:, base + off : base + off + 128]
                return bndB[:, qc * 256 + off : qc * 256 + off + 128]

            def qts_qc(qc):
                base = qc * (256 + NS) + 256
                return bndA[:, base : base + NS]

            def vts_vc(vc):
                base = QC * (256 + NS) + vc * kb2
                return bndA[:, base : base + kb2]

            def wv_hc(vc, hc):
                off = vc * 256 + (hc % 2) * 128
                if hc < 2:
                    base = QC * (256 + NS) + VC * kb2
                    return bndA[:, base + off : base + off + 128]
                return bndB[:, QC * 256 + off : QC * 256 + off + 128]

            # PE warmup: ramp the tensor-engine clock while DMAs land
            # (memset on the otherwise-idle Pool engine)
            warm = pp.tile([128, 256], BF16, name="warm")
            nc.gpsimd.memset(warm[:, :], 0.0)
            with tc.tile_pool(name="warmps", bufs=1, space="PSUM") as wps:
                pw = wps.tile([128, 256], F32, name="pw")
                for _ in range(10):
                    nc.tensor.matmul(pw[:, :], warm[:, 0:128], warm[:, :],
                                     start=True, stop=True)

            wlz0 = QC * 256 + VC * 256
            wlb = bndF[:, 0:12]
            msk = bndF[:, 12 : 12 + 2 * kb2]
            ident = bndF[:, 12 + 2 * kb2 :]

            # qpT: [128, (ph, b, d, ns)] bf16 (d = hc within phase)
            QPs = pp.tile([128, HC * NS], BF16, name="QPs")
            # duplicated-pair v-proj table: [128, (ph, i, b, d, 2)] bf16
            VP2 = pp.tile([128, HC * kb2 * 2], BF16, name="VP2")

            # logits psum: rows 0:kh <- strip 0, rows 32:32+kh <- strip 1
            ps_log = ppsum.tile([32 + kh, NS], F32, name="ps_log")

            proj_state = {}

            def proj_mm(ph):
                """PE half of phase-ph projection (v-proj first: its tiny
                matmuls burn the tensor engine's slow-clock window)."""
                ds = PHASES[ph]
                nd = len(ds)
                pool_cm = tc.tile_pool(name=f"p1ps{ph}", bufs=1, space="PSUM")
                p1ps = pool_cm.__enter__()
                pv = [p1ps.tile([128, kb2], F32, tag=f"pv{d}",
                                name="pv") for d in range(nd)]
                for vc in range(VC):
                    for d in range(nd):
                        nc.tensor.matmul(
                            pv[d][:, :],
                            wv_hc(vc, ds[d]),
                            vts_vc(vc),
                            start=(vc == 0),
                            stop=(vc == VC - 1),
                        )
                pq = [p1ps.tile([128, NS], F32, tag=f"pq{d}", name="pq")
                      for d in range(nd)]
                for qc in range(QC):
                    for d in range(nd):
                        nc.tensor.matmul(
                            pq[d][:, :],
                            wq_hc(qc, ds[d]),
                            qts_qc(qc),
                            start=(qc == 0),
                            stop=(qc == QC - 1),
                        )
                proj_state[ph] = (pool_cm, pv, pq)

            def vp2_copy(ph):
                """DVE: pv psum -> pair-duplicated VP2 (i, b, d, 2)."""
                ds = PHASES[ph]
                nd = len(ds)
                vp_v = VP2[:, voff[ph] : voff[ph] + nd * 2 * kb2].rearrange(
                    "p (i b dd two) -> p i b dd two", i=kpk, b=BPC, dd=nd
                )
                _, pv, _ = proj_state[ph]
                for d in range(nd):
                    hc = ds[d]
                    nc.vector.tensor_scalar_add(
                        vp_v[:, :, :, d : d + 1, :],
                        pv[d][:, :]
                        .rearrange("p (b i one two) -> p i b one two",
                                   b=BPC, one=1, two=1)
                        .broadcast_to((128, kpk, BPC, 1, 2)),
                        wlb[:, 2 * HC + hc : 2 * HC + hc + 1],
                    )

            def qps_copy(ph):
                """DVE: pq psum -> QPs bf16 (bq folded into VP2's bias);
                closes the phase's projection psum pool."""
                ds = PHASES[ph]
                nd = len(ds)
                qp_v = QPs[:, qoff[ph] : qoff[ph] + nd * NS].rearrange(
                    "p (b dd x) -> p b dd x", b=BPC, dd=nd
                )
                pool_cm, pv, pq = proj_state.pop(ph)
                for d in range(nd):
                    nc.vector.tensor_copy(
                        qp_v[:, :, d : d + 1, :],
                        pq[d][:, :].rearrange(
                            "p (b one x) -> p b one x", b=BPC, one=1
                        ),
                    )
                pool_cm.__exit__(None, None, None)

            def proj_copy(ph):
                vp2_copy(ph)
                qps_copy(ph)

            def main_phase(ph, mp, mid_cb=None, late_cb=None):
                """Joint tanh + logit matmuls for one phase's h-chunks."""
                ds = PHASES[ph]
                nd = len(ds)
                slabw = 2 * nd * NSB            # (b, d, x) cols per (j, strip)
                kind = 0 if ph == 0 else (2 if ph == nph - 1 else 1)
                groups = _groups(kh, kind)
                qp_ph = QPs[:, qoff[ph] : qoff[ph] + nd * NS]
                late_at = len(groups) - 1
                # phase 0: the first group reads the projection psum
                # directly (1x DVE mode) so the first tanh does not wait
                # for the QPs copies; the copies land at the g==1 boundary
                # (before the next phase's psum pool opens, keeping pool
                # LIFO order)
                psum_groups = 0
                mid_at = min(2, len(groups) - 1)
                for g, js in enumerate(groups):
                    if ph == 0 and g == 0:
                        qps_copy(0)
                    if g == mid_at and mid_cb is not None:
                        mid_cb()
                    if g == late_at and late_cb is not None:
                        late_cb()
                    L = len(js)
                    JT = mp.tile([128, L * 2 * slabw], BF16, tag="JT",
                                 name="JT")
                    for jj, j in enumerate(js):
                        for strip in range(2):
                            i0 = j + strip * kh
                            sb = (jj * 2 + strip) * slabw
                            vo = voff[ph] + i0 * 4 * nd
                            if g < psum_groups:
                                src0 = proj_state[0][2][0][:, :].rearrange(
                                    "p (bd xh two) -> p bd xh two",
                                    bd=2 * nd, xh=128,
                                )
                            else:
                                src0 = qp_ph.rearrange(
                                    "p (bd xh two) -> p bd xh two",
                                    bd=2 * nd, xh=128,
                                )
                            nc.vector.tensor_add(
                                JT[:, sb : sb + slabw].rearrange(
                                    "p (bd xh two) -> p bd xh two",
                                    bd=2 * nd, xh=128,
                                ),
                                src0,
                                VP2[:, vo : vo + 4 * nd]
                                .rearrange("p (bd one two) -> p bd one two",
                                           bd=2 * nd, one=1)
                                .broadcast_to((128, 2 * nd, 128, 2)),
                            )
                    # in-place tanh over the whole group
                    nc.scalar.activation(
                        JT[:, :], JT[:, :], mybir.ActivationFunctionType.Tanh
                    )
                    for jj, j in enumerate(js):
                        for d in range(nd):
                            hc = ds[d]
                            first = ph == 0 and g == 0 and jj == 0 and d == 0
                            last = (
                                ph == nph - 1
                                and g == len(groups) - 1
                                and jj == L - 1
                                and d == nd - 1
                            )
                            wl_col = bndB[
                                :,
                                wlz0 + hc * kh * kh + j * kh
                                : wlz0 + hc * kh * kh + (j + 1) * kh,
                            ]
                            for strip in range(2):
                                sb = (jj * 2 + strip) * slabw
                                rhs = JT[:, sb : sb + slabw].rearrange(
                                    "p (b dd x) -> p b dd x", b=BPC, dd=nd
                                )[:, :, d : d + 1, :]
                                nc.tensor.matmul(
                                    ps_log[32 * strip : 32 * strip + kh, :],
                                    wl_col,
                                    rhs,
                                    start=first,
                                    stop=last,
                                    tile_position=(0, 32 * strip),
                                    skip_group_check=True,
                                )

            def mk_mm_cb(p):
                def cb():
                    proj_mm(p)
                return cb

            def mk_copy_cb(p):
                def cb():
                    proj_copy(p)
                return cb

            proj_mm(0)
            vp2_copy(0)
            with tc.tile_pool(name="main", bufs=4) as mp:
                for ph in range(nph):
                    nxt = ph + 1 if ph + 1 < nph else None
                    main_phase(
                        ph, mp,
                        mid_cb=mk_mm_cb(nxt) if nxt is not None else None,
                        late_cb=mk_copy_cb(nxt) if nxt is not None else None,
                    )

            # ---- masked softmax over packed lanes ----
            LG0 = pp.tile([kh, NS], F32, name="LG0")
            LG1 = pp.tile([32 + kh, NS], F32, name="LG1")
            W_all = pp.tile([128, NS // 128, kpk], F32, name="W_all")
            nc.vector.tensor_copy(LG0[:, :], ps_log[0:kh, :])
            nc.vector.tensor_copy(LG1[32 : 32 + kh, :], ps_log[32 : 32 + kh, :])
            for nsc in range(NS // 128):
                b = nsc // (NSB // 128)
                LT = pp.tile([128, kpk], F32, name=f"LT{nsc}")
                for half in range(2):
                    ps_t = sps.tile([128, kh], F32, tag="ps_t", name="ps_t")
                    if half == 0:
                        src = LG0[0:kh, nsc * 128 : (nsc + 1) * 128]
                        idn = ident[0:kh, 0:kh]
                    else:
                        src = LG1[32 : 32 + kh, nsc * 128 : (nsc + 1) * 128]
                        idn = ident[32 : 32 + kh, 32 : 32 + kh]
                    nc.tensor.transpose(ps_t[:, :], src, idn)
                    nc.vector.tensor_copy(
                        LT[:, half * kh : (half + 1) * kh], ps_t[:, :]
                    )
                # masked = logits*validf + (validf-1)*1e9
                nc.vector.tensor_mul(
                    LT[:, :], LT[:, :], msk[:, b * kpk : (b + 1) * kpk]
                )
                nc.vector.tensor_add(
                    LT[:, :], LT[:, :], msk[:, kb2 + b * kpk : kb2 + (b + 1) * kpk]
                )
                # |logit| <= sum|Wl|*1 ~ 1.3, so exp needs no max-shift;
                # masked lanes are -1e9 -> exp underflows to exactly 0.
                EX = pp.tile([128, kpk], F32, name=f"EX{nsc}")
                sm = pp.tile([128, 1], F32, name=f"sm{nsc}")
                nc.scalar.activation(
                    EX[:, :], LT[:, :], mybir.ActivationFunctionType.Exp,
                    accum_out=sm[:, 0:1],
                )
                rs = pp.tile([128, 1], F32, name=f"rs{nsc}")
                nc.vector.reciprocal(rs[:, :], sm[:, :])
                nc.vector.tensor_scalar_mul(
                    W_all[:, nsc, :], EX[:, :], rs[:, 0:1]
                )
            # single output DMA: per-partition contiguous 512B descriptors
            # (per-nsc slices would cost 4x the descriptor count)
            nc.sync.dma_start(out_h[:, :], W_all[:, :, :])

    nc.finalize()
    return nc


def _prep_in_maps(v, q, box_mask, Wv, bv, Wq, bq, Wl, kpk, active):
    """Host-side layout prep: shard over B, pack active boxes, bundle."""
    import ml_dtypes

    kh = kpk // 2
    kb2 = BPC * kpk
    xa, xb = _bnd_layout(kpk)

    v = np.asarray(v, np.float32).reshape(B, K, VD)
    q = np.asarray(q, np.float32).reshape(B, N * S, QD)

    # packed v + validity per batch
    vpk = np.zeros((B, kpk, VD), np.float32)
    valid = np.zeros((B, kpk), np.float32)
    for b in range(B):
        kb = len(active[b])
        vpk[b, :kb] = v[b, active[b]]
        valid[b, :kb] = 1.0

    WqT = np.asarray(Wq, np.float32).T                                # [QD, H]
    WvT = np.asarray(Wv, np.float32).T                                # [VD, H]
    wl_chunks = np.asarray(Wl, np.float32).reshape(4, 128).T          # [128, hc]
    bvq = np.asarray(bv, np.float32) + np.asarray(bq, np.float32)

    # per-partition chunked views [128, C, x]
    def chunked(mT, width):  # mT [rows=c*128, width]
        c = mT.shape[0] // 128
        return mT.reshape(c, 128, width).transpose(1, 0, 2)           # [128,c,w]

    WqA = chunked(WqT[:, :256], 256)                                  # [128,6,256]
    WqB = chunked(WqT[:, 256:], 256)
    WvA = chunked(WvT[:, :256], 256)                                  # [128,8,256]
    WvB = chunked(WvT[:, 256:], 256)

    wlz = np.zeros((128, HC, kh, kh), np.float32)
    for j in range(kh):
        wlz[:, :, j, j] = wl_chunks
    wlz = wlz.reshape(128, HC * kh * kh)

    wlb = np.zeros((128, 12), np.float32)
    wlb[:, 0:4] = wl_chunks
    wlb[:, 8:12] = bvq.reshape(4, 128).T

    bndB = np.zeros((128, xb), np.float32)
    bndB[:, : QC * 256] = WqB.reshape(128, QC * 256)
    bndB[:, QC * 256 : QC * 256 + VC * 256] = WvB.reshape(128, VC * 256)
    bndB[:, QC * 256 + VC * 256 :] = wlz
    bndB = bndB.astype(ml_dtypes.bfloat16)

    ident = np.eye(128, dtype=np.float32)

    in_maps = []
    for c in range(NCORES):
        b0 = c * BPC
        qc = q[b0 : b0 + BPC].reshape(NS, QD)
        qTc = chunked(np.ascontiguousarray(qc.T), NS)                 # [128,6,NS]
        vc = vpk[b0 : b0 + BPC].reshape(kb2, VD)
        vTc = chunked(np.ascontiguousarray(vc.T), kb2)                # [128,8,kb2]

        bndA = np.zeros((128, xa), np.float32)
        for qq in range(QC):
            base = qq * (256 + NS)
            bndA[:, base : base + 256] = WqA[:, qq]
            bndA[:, base + 256 : base + 256 + NS] = qTc[:, qq]
        vo = QC * (256 + NS)
        bndA[:, vo : vo + VC * kb2] = vTc.reshape(128, VC * kb2)
        bndA[:, vo + VC * kb2 :] = WvA.reshape(128, VC * 256)
        bndA = bndA.astype(ml_dtypes.bfloat16)
        xa1 = 2 * (256 + NS)
        xa2 = (QC - 2) * (256 + NS)

        mf = valid[b0 : b0 + BPC].reshape(1, kb2)
        bndF = np.zeros((128, 12 + 2 * kb2 + 128), np.float32)
        bndF[:, 0:12] = wlb
        bndF[:, 12 : 12 + kb2] = mf
        bndF[:, 12 + kb2 : 12 + 2 * kb2] = (mf - 1.0) * 1e9
        bndF[:, 12 + 2 * kb2 :] = ident

        in_maps.append({
            "bndA1": np.ascontiguousarray(bndA[:, :xa1]),
            "bndA2": np.ascontiguousarray(bndA[:, xa1 : xa1 + xa2]),
            "bndA3": np.ascontiguousarray(bndA[:, xa1 + xa2 :]),
            "bndB": bndB,
            "bndF": bndF,
        })
    return in_maps


def kernel(v, q, box_mask, tags_attention, Wv, bv, Wq, bq, Wl, bl):
    # bl shifts all unmasked logits uniformly -> cancels in softmax.
    # tags_attention is unused by the reference module.
    bm = np.asarray(box_mask).reshape(B, K)
    active = [np.nonzero(bm[b] > 0)[0] for b in range(B)]
    kmax = max(len(a) for a in active)
    if kmax == 0:
        # every box masked in every batch: reference softmax is uniform
        return np.full((B, N, S, K), 1.0 / K, np.float32)
    kpk = max(2, kmax + (kmax & 1))       # even, >= 2

    if _CACHE.get("kpk") != kpk:
        _CACHE["nc"] = _build_nc(kpk)
        _CACHE["kpk"] = kpk
    nc = _CACHE["nc"]
    in_maps = _prep_in_maps(v, q, box_mask, Wv, bv, Wq, bq, Wl, kpk, active)
    res = bass_utils.run_bass_kernel_spmd(
        nc,
        in_maps,
        core_ids=list(range(NCORES)),
        trace=bool(os.environ.get("KERNEL_TRACE")),
        tmpdir=os.environ.get("KERNEL_TMPDIR"),
    )
    _CACHE["last_result"] = res
    w = np.zeros((B, N, S, K), np.float32)
    for c in range(NCORES):
        # out [128, (nsc, i)] -> rows (nsc*128+p) = (b, n, s) order
        wo = res.results[c]["out"].reshape(128, NS // 128, kpk)
        wp = wo.transpose(1, 0, 2).reshape(BPC, N, S, kpk)
        for bi in range(BPC):
            b = c * BPC + bi
            kb = len(active[b])
            if kb == 0:
                w[b] = 1.0 / K          # all-masked row: uniform softmax
            else:
                w[b][:, :, active[b]] = wp[bi][:, :, :kb]
    return w
